# revision 5
# baseline (speedup 1.0000x reference)
"""nn_AttentionConv_32487132627486 — Trainium2 Bass kernel (8 NeuronCores).

Sharding: data-parallel over batch — core i computes images [4i, 4i+4).
Each core runs a Bass/Tile kernel (built once at import, compiled lazily on
first call; the NEFF compile is served from the persistent compile cache).

Device algorithm (per core, partitions = (image-pair e, channel c)):
  Q/K/V 1x1-conv projections on TensorE (bf16, K/V written zero-padded);
  for each of the 49 window taps: P=(K_shift+rel)*Q on VectorE,
  group-sum+broadcast via a [128x128] selector matmul on TensorE,
  exp on ScalarE (PSUM->SBUF evict), softmax denominator via identity-matmul
  PSUM accumulation, attn*V on VectorE, output accumulation on GpSimd.
  Final normalize by 1/D and the adaptive mask, DMA out.
"""

import sys

for _p in ("/opt/trn_rl_repo",):
    if _p not in sys.path:
        sys.path.insert(0, _p)

import numpy as np
import ml_dtypes
import concourse.bacc as bacc
import concourse.mybir as mybir
import concourse.tile as tile

F32 = mybir.dt.float32
BF16 = mybir.dt.bfloat16
_BF16_NP = np.dtype(ml_dtypes.bfloat16)

B, CIN, CO, H, W, K, G, PAD = 32, 64, 64, 32, 32, 7, 8, 3
N_CORES = 8
B_C = B // N_CORES             # 4 images per core
HP = WP = H + 2 * PAD          # 38
NPIX = H * W                   # 1024
NQ = 2 * NPIX                  # 2048
KK = K * K


def _mm(nc, out, lhsT, rhs, start, stop, half=512):
    n = rhs.shape[-1]
    for s in range(0, n, half):
        e = min(s + half, n)
        nc.tensor.matmul(out[:, s:e], lhsT, rhs[:, s:e], start=start, stop=stop)


def _build_nc():
    nc = bacc.Bacc(None, debug=False)

    x_d = nc.declare_dram_parameter("xc", [CIN, B_C * NPIX], F32, isOutput=False)
    wq_d = nc.declare_dram_parameter("wqT", [CIN, CO], F32, isOutput=False)
    wk_d = nc.declare_dram_parameter("wkT", [CIN, CO], F32, isOutput=False)
    wv_d = nc.declare_dram_parameter("wvT", [CIN, CO], F32, isOutput=False)
    relcol_d = nc.declare_dram_parameter("relcol", [128, KK], F32, isOutput=False)
    sel_d = nc.declare_dram_parameter("sel128", [128, 128], F32, isOutput=False)
    eye_d = nc.declare_dram_parameter("eye128", [128, 128], F32, isOutput=False)
    mask_d = nc.declare_dram_parameter("maskbc", [128, NQ], F32, isOutput=False)
    out_d = nc.declare_dram_parameter("out", [128, NQ], F32, isOutput=True)

    with tile.TileContext(nc) as tc:
        with (
            tc.tile_pool(name="const", bufs=1) as constp,
            tc.tile_pool(name="proj", bufs=1) as projp,
            tc.tile_pool(name="psA", bufs=2, space="PSUM") as psA,
            tc.tile_pool(name="psD", bufs=1, space="PSUM") as psD,
            tc.tile_pool(name="ptile", bufs=3) as ptile,
            tc.tile_pool(name="atile", bufs=3) as atile,
            tc.tile_pool(name="otile", bufs=1) as otile,
        ):
            sel_sb = constp.tile([128, 128], F32, tag="sel")
            nc.gpsimd.dma_start(out=sel_sb[:], in_=sel_d[:])
            eye_sb = constp.tile([128, 128], F32, tag="eye")
            nc.gpsimd.dma_start(out=eye_sb[:], in_=eye_d[:])
            relcol_sb = constp.tile([128, KK], F32, tag="rel")
            nc.gpsimd.dma_start(out=relcol_sb[:], in_=relcol_d[:])
            mask_sb = constp.tile([128, NQ], F32, tag="mask")
            nc.gpsimd.dma_start(out=mask_sb[:], in_=mask_d[:])

            x_sb = projp.tile([CIN, B_C * NPIX], F32, tag="x")
            nc.sync.dma_start(out=x_sb[:], in_=x_d[:])
            wq_sb = constp.tile([CIN, CO], F32, tag="w1")
            wk_sb = constp.tile([CIN, CO], F32, tag="w2")
            wv_sb = constp.tile([CIN, CO], F32, tag="w3")
            nc.gpsimd.dma_start(out=wq_sb[:], in_=wq_d[:])
            nc.gpsimd.dma_start(out=wk_sb[:], in_=wk_d[:])
            nc.gpsimd.dma_start(out=wv_sb[:], in_=wv_d[:])

            q_sb = projp.tile([128, NQ], F32, tag="q")
            k_sb = projp.tile([128, 2, HP, WP], F32, tag="k")
            v_sb = projp.tile([128, 2, HP, WP], F32, tag="v")
            nc.vector.memset(k_sb[:], 0.0)
            nc.vector.memset(v_sb[:], 0.0)

            for (wt, dst, padded) in ((wq_sb, q_sb, False), (wk_sb, k_sb, True),
                                      (wv_sb, v_sb, True)):
                for e in range(2):
                    for ch in range(2):
                        pp = psA.tile([128, NPIX], F32, tag="sps")
                        img = 2 * e + ch
                        _mm(nc, pp[64 * e:64 * e + 64, :], wt[:],
                            x_sb[:, NPIX * img: NPIX * img + NPIX],
                            start=True, stop=True)
                        src = pp[64 * e:64 * e + 64].rearrange(
                            "p (h w) -> p h w", h=H)
                        if padded:
                            dv = dst[64 * e:64 * e + 64, ch,
                                     PAD:PAD + H, PAD:PAD + W]
                        else:
                            dv = dst[64 * e:64 * e + 64,
                                     NPIX * ch:NPIX * ch + NPIX].rearrange(
                                "p (h w) -> p h w", h=H)
                        nc.scalar.activation(
                            out=dv, in_=src,
                            func=mybir.ActivationFunctionType.Copy)

            o_sb = otile.tile([128, NQ], F32, tag="o")
            dm_sb = otile.tile([128, NQ], F32, tag="dm")
            for qc in range(2):
                d_ps = psD.tile([128, NPIX], F32, tag="dps")
                o_chunk = o_sb[:, NPIX * qc:NPIX * qc + NPIX].rearrange(
                    "p (h w) -> p h w", h=H)
                q_chunk = q_sb[:, NPIX * qc:NPIX * qc + NPIX].rearrange(
                    "p (h w) -> p h w", h=H)
                for t in range(KK):
                    dh, dw = t // K, t % K
                    kview = k_sb[:, qc, dh:dh + H, dw:dw + W]
                    vview = v_sb[:, qc, dh:dh + H, dw:dw + W]
                    p_t = ptile.tile([128, H, W], F32, tag="pt")
                    nc.vector.scalar_tensor_tensor(
                        out=p_t[:], in0=kview, scalar=relcol_sb[:, t:t + 1],
                        in1=q_chunk,
                        op0=mybir.AluOpType.add, op1=mybir.AluOpType.mult)
                    s_ps = psA.tile([128, NPIX], F32, tag="sps")
                    _mm(nc, s_ps[:], sel_sb[:],
                        p_t.rearrange("p h w -> p (h w)"),
                        start=True, stop=True)
                    a_t = atile.tile([128, NPIX], F32, tag="at")
                    nc.scalar.activation(out=a_t[:], in_=s_ps[:],
                                         func=mybir.ActivationFunctionType.Exp)
                    _mm(nc, d_ps[:], eye_sb[:], a_t[:],
                        start=(t == 0), stop=(t == KK - 1))
                    p2_t = ptile.tile([128, H, W], F32, tag="p2t")
                    nc.vector.tensor_tensor(
                        out=p2_t[:], in0=a_t.rearrange("p (h w) -> p h w", h=H),
                        in1=vview, op=mybir.AluOpType.mult)
                    if t == 0:
                        nc.gpsimd.tensor_copy(out=o_chunk, in_=p2_t[:])
                    else:
                        nc.gpsimd.tensor_tensor(out=o_chunk, in0=o_chunk,
                                                in1=p2_t[:],
                                                op=mybir.AluOpType.add)
                with nc.allow_low_precision(reason="softmax denom recip, tol 2e-2"):
                    nc.vector.reciprocal(
                        out=dm_sb[:, NPIX * qc:NPIX * qc + NPIX], in_=d_ps[:])

            nc.vector.tensor_tensor(out=dm_sb[:], in0=dm_sb[:], in1=mask_sb[:],
                                    op=mybir.AluOpType.mult)
            of = otile.tile([128, NQ], F32, tag="ofin")
            nc.vector.tensor_tensor(out=of[:], in0=o_sb[:], in1=dm_sb[:],
                                    op=mybir.AluOpType.mult)
            nc.sync.dma_start(out=out_d[:], in_=of[:])

    nc.finalize()
    return nc


def _make_host_consts(w_q, w_k, w_v, rel_h, rel_w, current_val):
    wqT = np.ascontiguousarray(np.asarray(w_q, np.float32).T).astype(np.float32)
    wkT = np.ascontiguousarray(np.asarray(w_k, np.float32).T).astype(np.float32)
    wvT = np.ascontiguousarray(np.asarray(w_v, np.float32).T).astype(np.float32)
    rh = np.asarray(rel_h, np.float32).reshape(32, K)   # [c<32, dh]
    rw = np.asarray(rel_w, np.float32).reshape(32, K)   # [c>=32, dw]
    relcol = np.zeros((128, KK), np.float32)
    for t in range(KK):
        dh, dw = t // K, t % K
        col = np.concatenate([rh[:, dh], rw[:, dw]])
        relcol[:, t] = np.tile(col, 2)
    ee = np.arange(128) // 64
    cc = np.arange(128) % 64
    gg = cc // (CO // G)
    sel128 = ((ee[:, None] == ee[None, :]) &
              (gg[:, None] == gg[None, :])).astype(np.float32)
    eye128 = np.eye(128).astype(np.float32)

    MAXSZ = W // 2
    template = np.linspace(1.0 - MAXSZ, 0.0, MAXSZ).astype(np.float32)
    om = (template[None, :]
          + np.asarray(current_val, np.float32) * MAXSZ) / 3.0 + 1.0
    om = np.clip(om, 0.0, 1.0)
    i = np.arange(W)
    r = np.minimum(i, W - 1 - i)
    top = i <= (W - 1 - i)
    lo = np.where(top, r, r + 1)
    hi = W - 1 - r
    c = np.arange(W)
    in_ring = (c[None, :] >= lo[:, None]) & (c[None, :] <= hi[:, None])
    vals = om[:, r]
    mask = np.where(in_ring[None], vals[:, :, None], 1.0).astype(np.float32)
    maskbc = np.zeros((128, NQ), np.float32)
    for p in range(128):
        g = (p % 64) // (CO // G)
        maskbc[p] = np.tile(mask[g].reshape(-1), 2)
    return dict(wqT=wqT, wkT=wkT, wvT=wvT, relcol=relcol, sel128=sel128,
                eye128=eye128, maskbc=maskbc.astype(np.float32))


_NC = None


def _get_nc():
    global _NC
    if _NC is None:
        _NC = _build_nc()
    return _NC


def kernel(x, w_q, w_k, w_v, rel_h, rel_w, current_val):
    from concourse.bass_utils import run_bass_kernel_spmd

    x = np.asarray(x, np.float32)
    nc = _get_nc()
    consts = _make_host_consts(w_q, w_k, w_v, rel_h, rel_w, current_val)
    in_maps = []
    for i in range(N_CORES):
        x4 = x[B_C * i:B_C * i + B_C]
        xc = np.ascontiguousarray(
            x4.transpose(1, 0, 2, 3).reshape(CIN, -1)).astype(np.float32)
        m = dict(consts)
        m["xc"] = xc
        in_maps.append(m)

    res = run_bass_kernel_spmd(nc, in_maps, core_ids=list(range(N_CORES)))

    outs = []
    for i in range(N_CORES):
        o = np.asarray(res.results[i]["out"], np.float32).reshape(2, 64, 2, H, W)
        outs.append(o.transpose(0, 2, 1, 3, 4).reshape(B_C, CO, H, W))
    full = np.ascontiguousarray(np.concatenate(outs, axis=0)).astype(np.float32)
    # reference returns grouped shape [B, G, CPG, H, W]
    return full.reshape(B, G, CO // G, H, W)


# Build (and, via the persistent compile cache, warm) at import so that the
# first timed kernel() call does not pay IR construction.
_get_nc()


# revision 6
# speedup vs baseline: 1.5183x; 1.5183x over previous
"""nn_AttentionConv_32487132627486 — Trainium2 Bass kernel (8 NeuronCores).

Sharding: data-parallel over batch — core i computes images [4i, 4i+4).
Each core runs a Bass/Tile kernel (built once at import, compiled lazily on
first call; the NEFF compile is served from the persistent compile cache).

Device algorithm (per core, partitions = (image-pair e, channel c)):
  Q/K/V 1x1-conv projections on TensorE (bf16, K/V written zero-padded);
  for each of the 49 window taps: P=(K_shift+rel)*Q on VectorE,
  group-sum+broadcast via a [128x128] selector matmul on TensorE,
  exp on ScalarE (PSUM->SBUF evict), softmax denominator via identity-matmul
  PSUM accumulation, attn*V on VectorE, output accumulation on GpSimd.
  Final normalize by 1/D and the adaptive mask, DMA out.
"""

import sys

for _p in ("/opt/trn_rl_repo",):
    if _p not in sys.path:
        sys.path.insert(0, _p)

import numpy as np
import ml_dtypes
import concourse.bacc as bacc
import concourse.mybir as mybir
import concourse.tile as tile

F32 = mybir.dt.float32
BF16 = mybir.dt.bfloat16
_BF16_NP = np.dtype(ml_dtypes.bfloat16)

B, CIN, CO, H, W, K, G, PAD = 32, 64, 64, 32, 32, 7, 8, 3
N_CORES = 8
B_C = B // N_CORES             # 4 images per core
HP = WP = H + 2 * PAD          # 38
NPIX = H * W                   # 1024
NQ = 2 * NPIX                  # 2048
KK = K * K


def _mm(nc, out, lhsT, rhs, start, stop, half=512):
    n = rhs.shape[-1]
    for s in range(0, n, half):
        e = min(s + half, n)
        nc.tensor.matmul(out[:, s:e], lhsT, rhs[:, s:e], start=start, stop=stop)


def _build_nc():
    nc = bacc.Bacc(None, debug=False)

    x_d = nc.declare_dram_parameter("xc", [CIN, B_C * NPIX], F32, isOutput=False)
    wq_d = nc.declare_dram_parameter("wqT", [CIN, CO], F32, isOutput=False)
    wk_d = nc.declare_dram_parameter("wkT", [CIN, CO], F32, isOutput=False)
    wv_d = nc.declare_dram_parameter("wvT", [CIN, CO], F32, isOutput=False)
    relcol_d = nc.declare_dram_parameter("relcol", [128, KK], F32, isOutput=False)
    sel_d = nc.declare_dram_parameter("sel128", [128, 128], F32, isOutput=False)
    eye_d = nc.declare_dram_parameter("eye128", [128, 128], F32, isOutput=False)
    out_d = nc.declare_dram_parameter("out", [128, NQ], BF16, isOutput=True)

    with tile.TileContext(nc) as tc:
        with (
            tc.tile_pool(name="const", bufs=1) as constp,
            tc.tile_pool(name="proj", bufs=1) as projp,
            tc.tile_pool(name="psA", bufs=2, space="PSUM") as psA,
            tc.tile_pool(name="psD", bufs=1, space="PSUM") as psD,
            tc.tile_pool(name="ptile", bufs=3) as ptile,
            tc.tile_pool(name="atile", bufs=3) as atile,
            tc.tile_pool(name="otile", bufs=1) as otile,
        ):
            sel_sb = constp.tile([128, 128], F32, tag="sel")
            nc.gpsimd.dma_start(out=sel_sb[:], in_=sel_d[:])
            eye_sb = constp.tile([128, 128], F32, tag="eye")
            nc.gpsimd.dma_start(out=eye_sb[:], in_=eye_d[:])
            relcol_sb = constp.tile([128, KK], F32, tag="rel")
            nc.gpsimd.dma_start(out=relcol_sb[:], in_=relcol_d[:])

            x_sb = projp.tile([CIN, B_C * NPIX], F32, tag="x")
            nc.sync.dma_start(out=x_sb[:], in_=x_d[:])
            wq_sb = constp.tile([CIN, CO], F32, tag="w1")
            wk_sb = constp.tile([CIN, CO], F32, tag="w2")
            wv_sb = constp.tile([CIN, CO], F32, tag="w3")
            nc.gpsimd.dma_start(out=wq_sb[:], in_=wq_d[:])
            nc.gpsimd.dma_start(out=wk_sb[:], in_=wk_d[:])
            nc.gpsimd.dma_start(out=wv_sb[:], in_=wv_d[:])

            q_sb = projp.tile([128, NQ], F32, tag="q")
            k_sb = projp.tile([128, 2, HP, WP], F32, tag="k")
            v_sb = projp.tile([128, 2, HP, WP], F32, tag="v")
            nc.vector.memset(k_sb[:], 0.0)
            nc.vector.memset(v_sb[:], 0.0)

            for (wt, dst, padded) in ((wq_sb, q_sb, False), (wk_sb, k_sb, True),
                                      (wv_sb, v_sb, True)):
                for e in range(2):
                    for ch in range(2):
                        pp = psA.tile([128, NPIX], F32, tag="sps")
                        img = 2 * e + ch
                        _mm(nc, pp[64 * e:64 * e + 64, :], wt[:],
                            x_sb[:, NPIX * img: NPIX * img + NPIX],
                            start=True, stop=True)
                        src = pp[64 * e:64 * e + 64].rearrange(
                            "p (h w) -> p h w", h=H)
                        if padded:
                            dv = dst[64 * e:64 * e + 64, ch,
                                     PAD:PAD + H, PAD:PAD + W]
                        else:
                            dv = dst[64 * e:64 * e + 64,
                                     NPIX * ch:NPIX * ch + NPIX].rearrange(
                                "p (h w) -> p h w", h=H)
                        nc.scalar.activation(
                            out=dv, in_=src,
                            func=mybir.ActivationFunctionType.Copy)

            o_sb = otile.tile([128, NQ], F32, tag="o")
            dm_sb = otile.tile([128, NQ], F32, tag="dm")
            for qc in range(2):
                d_ps = psD.tile([128, NPIX], F32, tag="dps")
                o_chunk = o_sb[:, NPIX * qc:NPIX * qc + NPIX].rearrange(
                    "p (h w) -> p h w", h=H)
                q_chunk = q_sb[:, NPIX * qc:NPIX * qc + NPIX].rearrange(
                    "p (h w) -> p h w", h=H)
                for t in range(KK):
                    dh, dw = t // K, t % K
                    kview = k_sb[:, qc, dh:dh + H, dw:dw + W]
                    vview = v_sb[:, qc, dh:dh + H, dw:dw + W]
                    p_t = ptile.tile([128, H, W], F32, tag="pt")
                    nc.vector.scalar_tensor_tensor(
                        out=p_t[:], in0=kview, scalar=relcol_sb[:, t:t + 1],
                        in1=q_chunk,
                        op0=mybir.AluOpType.add, op1=mybir.AluOpType.mult)
                    s_ps = psA.tile([128, NPIX], F32, tag="sps")
                    _mm(nc, s_ps[:], sel_sb[:],
                        p_t.rearrange("p h w -> p (h w)"),
                        start=True, stop=True)
                    a_t = atile.tile([128, NPIX], F32, tag="at")
                    nc.scalar.activation(out=a_t[:], in_=s_ps[:],
                                         func=mybir.ActivationFunctionType.Exp)
                    _mm(nc, d_ps[:], eye_sb[:], a_t[:],
                        start=(t == 0), stop=(t == KK - 1))
                    p2_t = ptile.tile([128, H, W], F32, tag="p2t")
                    nc.vector.tensor_tensor(
                        out=p2_t[:], in0=a_t.rearrange("p (h w) -> p h w", h=H),
                        in1=vview, op=mybir.AluOpType.mult)
                    if t == 0:
                        nc.gpsimd.tensor_copy(out=o_chunk, in_=p2_t[:])
                    else:
                        nc.gpsimd.tensor_tensor(out=o_chunk, in0=o_chunk,
                                                in1=p2_t[:],
                                                op=mybir.AluOpType.add)
                with nc.allow_low_precision(reason="softmax denom recip, tol 2e-2"):
                    nc.vector.reciprocal(
                        out=dm_sb[:, NPIX * qc:NPIX * qc + NPIX], in_=d_ps[:])

            of = otile.tile([128, NQ], BF16, tag="ofin")
            nc.vector.tensor_tensor(out=of[:], in0=o_sb[:], in1=dm_sb[:],
                                    op=mybir.AluOpType.mult)
            nc.sync.dma_start(out=out_d[:], in_=of[:])

    nc.finalize()
    return nc


def _make_host_consts(w_q, w_k, w_v, rel_h, rel_w, current_val):
    wqT = np.ascontiguousarray(np.asarray(w_q, np.float32).T).astype(np.float32)
    wkT = np.ascontiguousarray(np.asarray(w_k, np.float32).T).astype(np.float32)
    wvT = np.ascontiguousarray(np.asarray(w_v, np.float32).T).astype(np.float32)
    rh = np.asarray(rel_h, np.float32).reshape(32, K)   # [c<32, dh]
    rw = np.asarray(rel_w, np.float32).reshape(32, K)   # [c>=32, dw]
    relcol = np.zeros((128, KK), np.float32)
    for t in range(KK):
        dh, dw = t // K, t % K
        col = np.concatenate([rh[:, dh], rw[:, dw]])
        relcol[:, t] = np.tile(col, 2)
    ee = np.arange(128) // 64
    cc = np.arange(128) % 64
    gg = cc // (CO // G)
    sel128 = ((ee[:, None] == ee[None, :]) &
              (gg[:, None] == gg[None, :])).astype(np.float32)
    eye128 = np.eye(128).astype(np.float32)

    MAXSZ = W // 2
    template = np.linspace(1.0 - MAXSZ, 0.0, MAXSZ).astype(np.float32)
    om = (template[None, :]
          + np.asarray(current_val, np.float32) * MAXSZ) / 3.0 + 1.0
    om = np.clip(om, 0.0, 1.0)
    i = np.arange(W)
    r = np.minimum(i, W - 1 - i)
    top = i <= (W - 1 - i)
    lo = np.where(top, r, r + 1)
    hi = W - 1 - r
    c = np.arange(W)
    in_ring = (c[None, :] >= lo[:, None]) & (c[None, :] <= hi[:, None])
    vals = om[:, r]
    mask = np.where(in_ring[None], vals[:, :, None], 1.0).astype(np.float32)
    return dict(wqT=wqT, wkT=wkT, wvT=wvT, relcol=relcol, sel128=sel128,
                eye128=eye128), mask


_NC = None


def _get_nc():
    global _NC
    if _NC is None:
        _NC = _build_nc()
    return _NC


def kernel(x, w_q, w_k, w_v, rel_h, rel_w, current_val):
    from concourse.bass_utils import run_bass_kernel_spmd

    x = np.asarray(x, np.float32)
    nc = _get_nc()
    consts, mask = _make_host_consts(w_q, w_k, w_v, rel_h, rel_w, current_val)
    in_maps = []
    for i in range(N_CORES):
        x4 = x[B_C * i:B_C * i + B_C]
        xc = np.ascontiguousarray(
            x4.transpose(1, 0, 2, 3).reshape(CIN, -1)).astype(np.float32)
        m = dict(consts)
        m["xc"] = xc
        in_maps.append(m)

    res = run_bass_kernel_spmd(nc, in_maps, core_ids=list(range(N_CORES)))

    outs = []
    for i in range(N_CORES):
        o = np.asarray(res.results[i]["out"], np.float32).reshape(2, 64, 2, H, W)
        outs.append(o.transpose(0, 2, 1, 3, 4).reshape(B_C, CO, H, W))
    full = np.ascontiguousarray(np.concatenate(outs, axis=0)).astype(np.float32)
    # reference returns grouped shape [B, G, CPG, H, W]; adaptive mask on host
    out5 = full.reshape(B, G, CO // G, H, W)
    return (out5 * mask[None, :, None, :, :]).astype(np.float32)


# Build (and, via the persistent compile cache, warm) at import so that the
# first timed kernel() call does not pay IR construction.
_get_nc()


# revision 8
# speedup vs baseline: 1.5542x; 1.0237x over previous
"""nn_AttentionConv_32487132627486 — Trainium2 Bass kernel (8 NeuronCores).

Sharding: data-parallel over batch — core i computes images [4i, 4i+4).
Each core runs a Bass/Tile kernel (built once at import, compiled lazily on
first call; the NEFF compile is served from the persistent compile cache).

Device algorithm (per core, partitions = (image-pair e, channel c)):
  Q/K/V 1x1-conv projections on TensorE (bf16, K/V written zero-padded);
  for each of the 49 window taps: P=(K_shift+rel)*Q on VectorE,
  group-sum+broadcast via a [128x128] selector matmul on TensorE,
  exp on ScalarE (PSUM->SBUF evict), softmax denominator via identity-matmul
  PSUM accumulation, attn*V on VectorE, output accumulation on GpSimd.
  Final normalize by 1/D and the adaptive mask, DMA out.
"""

import sys

for _p in ("/opt/trn_rl_repo",):
    if _p not in sys.path:
        sys.path.insert(0, _p)

import numpy as np
import ml_dtypes
import concourse.bacc as bacc
import concourse.mybir as mybir
import concourse.tile as tile

F32 = mybir.dt.float32
BF16 = mybir.dt.bfloat16
_BF16_NP = np.dtype(ml_dtypes.bfloat16)

B, CIN, CO, H, W, K, G, PAD = 32, 64, 64, 32, 32, 7, 8, 3
N_CORES = 8
B_C = B // N_CORES             # 4 images per core
HP = WP = H + 2 * PAD          # 38
NPIX = H * W                   # 1024
NQ = 2 * NPIX                  # 2048
KK = K * K


def _mm(nc, out, lhsT, rhs, start, stop, half=512):
    n = rhs.shape[-1]
    for s in range(0, n, half):
        e = min(s + half, n)
        nc.tensor.matmul(out[:, s:e], lhsT, rhs[:, s:e], start=start, stop=stop)


def _build_nc():
    nc = bacc.Bacc(None, debug=False)

    x_d = nc.declare_dram_parameter("xc", [CIN, B_C * NPIX], F32, isOutput=False)
    wq_d = nc.declare_dram_parameter("wqT", [CIN, CO], F32, isOutput=False)
    wk_d = nc.declare_dram_parameter("wkT", [CIN, CO], F32, isOutput=False)
    wv_d = nc.declare_dram_parameter("wvT", [CIN, CO], F32, isOutput=False)
    relcol_d = nc.declare_dram_parameter("relcol", [128, KK], F32, isOutput=False)
    sel_d = nc.declare_dram_parameter("sel128", [128, 128], F32, isOutput=False)
    out_d = nc.declare_dram_parameter("out", [128, NQ], BF16, isOutput=True)

    with tile.TileContext(nc) as tc:
        with (
            tc.tile_pool(name="const", bufs=1) as constp,
            tc.tile_pool(name="proj", bufs=1) as projp,
            tc.tile_pool(name="psA", bufs=2, space="PSUM") as psA,
            tc.tile_pool(name="ptile", bufs=3) as ptile,
            tc.tile_pool(name="atile", bufs=3) as atile,
            tc.tile_pool(name="otile", bufs=1) as otile,
        ):
            sel_sb = constp.tile([128, 128], F32, tag="sel")
            nc.gpsimd.dma_start(out=sel_sb[:], in_=sel_d[:])
            relcol_sb = constp.tile([128, KK], F32, tag="rel")
            nc.gpsimd.dma_start(out=relcol_sb[:], in_=relcol_d[:])

            x_sb = projp.tile([CIN, B_C * NPIX], F32, tag="x")
            nc.sync.dma_start(out=x_sb[:], in_=x_d[:])
            wq_sb = constp.tile([CIN, CO], F32, tag="w1")
            wk_sb = constp.tile([CIN, CO], F32, tag="w2")
            wv_sb = constp.tile([CIN, CO], F32, tag="w3")
            nc.gpsimd.dma_start(out=wq_sb[:], in_=wq_d[:])
            nc.gpsimd.dma_start(out=wk_sb[:], in_=wk_d[:])
            nc.gpsimd.dma_start(out=wv_sb[:], in_=wv_d[:])

            q_sb = projp.tile([128, NQ], F32, tag="q")
            k_sb = projp.tile([128, 2, HP, WP], F32, tag="k")
            v_sb = projp.tile([128, 2, HP, WP], F32, tag="v")
            nc.vector.memset(k_sb[:], 0.0)
            nc.vector.memset(v_sb[:], 0.0)

            for (wt, dst, padded) in ((wq_sb, q_sb, False), (wk_sb, k_sb, True),
                                      (wv_sb, v_sb, True)):
                for e in range(2):
                    for ch in range(2):
                        pp = psA.tile([128, NPIX], F32, tag="sps")
                        img = 2 * e + ch
                        _mm(nc, pp[64 * e:64 * e + 64, :], wt[:],
                            x_sb[:, NPIX * img: NPIX * img + NPIX],
                            start=True, stop=True)
                        src = pp[64 * e:64 * e + 64].rearrange(
                            "p (h w) -> p h w", h=H)
                        if padded:
                            dv = dst[64 * e:64 * e + 64, ch,
                                     PAD:PAD + H, PAD:PAD + W]
                        else:
                            dv = dst[64 * e:64 * e + 64,
                                     NPIX * ch:NPIX * ch + NPIX].rearrange(
                                "p (h w) -> p h w", h=H)
                        nc.scalar.activation(
                            out=dv, in_=src,
                            func=mybir.ActivationFunctionType.Copy)

            o_sb = otile.tile([128, NQ], F32, tag="o")
            d_sb = otile.tile([128, NQ], F32, tag="d")
            dm_sb = otile.tile([128, NQ], F32, tag="dm")
            for t in range(KK):
                dh, dw = t // K, t % K
                a_t = atile.tile([128, NQ], F32, tag="at")
                p2_t = ptile.tile([128, NQ], F32, tag="p2t")
                for qc in range(2):
                    kview = k_sb[:, qc, dh:dh + H, dw:dw + W]
                    vview = v_sb[:, qc, dh:dh + H, dw:dw + W]
                    q_chunk = q_sb[:, NPIX * qc:NPIX * qc + NPIX].rearrange(
                        "p (h w) -> p h w", h=H)
                    p_t = ptile.tile([128, H, W], F32, tag="pt")
                    nc.vector.scalar_tensor_tensor(
                        out=p_t[:], in0=kview, scalar=relcol_sb[:, t:t + 1],
                        in1=q_chunk,
                        op0=mybir.AluOpType.add, op1=mybir.AluOpType.mult)
                    s_ps = psA.tile([128, NPIX], F32, tag="sps")
                    _mm(nc, s_ps[:], sel_sb[:],
                        p_t.rearrange("p h w -> p (h w)"),
                        start=True, stop=True)
                    a_c = a_t[:, NPIX * qc:NPIX * qc + NPIX]
                    nc.scalar.activation(out=a_c, in_=s_ps[:],
                                         func=mybir.ActivationFunctionType.Exp)
                    nc.vector.tensor_tensor(
                        out=p2_t[:, NPIX * qc:NPIX * qc + NPIX].rearrange(
                            "p (h w) -> p h w", h=H),
                        in0=a_c.rearrange("p (h w) -> p h w", h=H),
                        in1=vview, op=mybir.AluOpType.mult)
                if t == 0:
                    nc.gpsimd.tensor_copy(out=d_sb[:], in_=a_t[:])
                    nc.vector.tensor_copy(out=o_sb[:], in_=p2_t[:])
                else:
                    nc.gpsimd.tensor_tensor(out=d_sb[:], in0=d_sb[:],
                                            in1=a_t[:],
                                            op=mybir.AluOpType.add)
                    nc.vector.tensor_tensor(out=o_sb[:], in0=o_sb[:],
                                            in1=p2_t[:],
                                            op=mybir.AluOpType.add)
            with nc.allow_low_precision(reason="softmax denom recip, tol 2e-2"):
                nc.vector.reciprocal(out=dm_sb[:], in_=d_sb[:])

            of = otile.tile([128, NQ], BF16, tag="ofin")
            nc.vector.tensor_tensor(out=of[:], in0=o_sb[:], in1=dm_sb[:],
                                    op=mybir.AluOpType.mult)
            nc.sync.dma_start(out=out_d[:], in_=of[:])

    nc.finalize()
    return nc


def _make_host_consts(w_q, w_k, w_v, rel_h, rel_w, current_val):
    wqT = np.ascontiguousarray(np.asarray(w_q, np.float32).T).astype(np.float32)
    wkT = np.ascontiguousarray(np.asarray(w_k, np.float32).T).astype(np.float32)
    wvT = np.ascontiguousarray(np.asarray(w_v, np.float32).T).astype(np.float32)
    rh = np.asarray(rel_h, np.float32).reshape(32, K)   # [c<32, dh]
    rw = np.asarray(rel_w, np.float32).reshape(32, K)   # [c>=32, dw]
    relcol = np.zeros((128, KK), np.float32)
    for t in range(KK):
        dh, dw = t // K, t % K
        col = np.concatenate([rh[:, dh], rw[:, dw]])
        relcol[:, t] = np.tile(col, 2)
    ee = np.arange(128) // 64
    cc = np.arange(128) % 64
    gg = cc // (CO // G)
    sel128 = ((ee[:, None] == ee[None, :]) &
              (gg[:, None] == gg[None, :])).astype(np.float32)

    MAXSZ = W // 2
    template = np.linspace(1.0 - MAXSZ, 0.0, MAXSZ).astype(np.float32)
    om = (template[None, :]
          + np.asarray(current_val, np.float32) * MAXSZ) / 3.0 + 1.0
    om = np.clip(om, 0.0, 1.0)
    i = np.arange(W)
    r = np.minimum(i, W - 1 - i)
    top = i <= (W - 1 - i)
    lo = np.where(top, r, r + 1)
    hi = W - 1 - r
    c = np.arange(W)
    in_ring = (c[None, :] >= lo[:, None]) & (c[None, :] <= hi[:, None])
    vals = om[:, r]
    mask = np.where(in_ring[None], vals[:, :, None], 1.0).astype(np.float32)
    return dict(wqT=wqT, wkT=wkT, wvT=wvT, relcol=relcol,
                sel128=sel128), mask


_NC = None


def _get_nc():
    global _NC
    if _NC is None:
        _NC = _build_nc()
    return _NC


def kernel(x, w_q, w_k, w_v, rel_h, rel_w, current_val):
    from concourse.bass_utils import run_bass_kernel_spmd

    x = np.asarray(x, np.float32)
    nc = _get_nc()
    consts, mask = _make_host_consts(w_q, w_k, w_v, rel_h, rel_w, current_val)
    in_maps = []
    for i in range(N_CORES):
        x4 = x[B_C * i:B_C * i + B_C]
        xc = np.ascontiguousarray(
            x4.transpose(1, 0, 2, 3).reshape(CIN, -1)).astype(np.float32)
        m = dict(consts)
        m["xc"] = xc
        in_maps.append(m)

    res = run_bass_kernel_spmd(nc, in_maps, core_ids=list(range(N_CORES)))

    outs = []
    for i in range(N_CORES):
        o = np.asarray(res.results[i]["out"], np.float32).reshape(2, 64, 2, H, W)
        outs.append(o.transpose(0, 2, 1, 3, 4).reshape(B_C, CO, H, W))
    full = np.ascontiguousarray(np.concatenate(outs, axis=0)).astype(np.float32)
    # reference returns grouped shape [B, G, CPG, H, W]; adaptive mask on host
    out5 = full.reshape(B, G, CO // G, H, W)
    return (out5 * mask[None, :, None, :, :]).astype(np.float32)


# Build (and, via the persistent compile cache, warm) at import so that the
# first timed kernel() call does not pay IR construction.
_get_nc()


# revision 9
# speedup vs baseline: 2.3750x; 1.5281x over previous
"""nn_AttentionConv_32487132627486 — Trainium2 Bass kernel (8 NeuronCores).

Sharding: data-parallel over batch — core i computes images [4i, 4i+4).
Each core runs a Bass/Tile kernel (built once at import, compiled lazily on
first call; the NEFF compile is served from the persistent compile cache).

Device algorithm (per core, partitions = (image-pair e, channel c)):
  Q/K/V 1x1-conv projections on TensorE (bf16, K/V written zero-padded);
  for each of the 49 window taps: P=(K_shift+rel)*Q on VectorE,
  group-sum+broadcast via a [128x128] selector matmul on TensorE,
  exp on ScalarE (PSUM->SBUF evict), softmax denominator via identity-matmul
  PSUM accumulation, attn*V on VectorE, output accumulation on GpSimd.
  Final normalize by 1/D and the adaptive mask, DMA out.
"""

import sys

for _p in ("/opt/trn_rl_repo",):
    if _p not in sys.path:
        sys.path.insert(0, _p)

import numpy as np
import ml_dtypes
import concourse.bacc as bacc
import concourse.mybir as mybir
import concourse.tile as tile

F32 = mybir.dt.float32
BF16 = mybir.dt.bfloat16
_BF16_NP = np.dtype(ml_dtypes.bfloat16)

B, CIN, CO, H, W, K, G, PAD = 32, 64, 64, 32, 32, 7, 8, 3
N_CORES = 8
B_C = B // N_CORES             # 4 images per core
HP = WP = H + 2 * PAD          # 38
NPIX = H * W                   # 1024
NQ = 2 * NPIX                  # 2048
KK = K * K


def _mm(nc, out, lhsT, rhs, start, stop, half=512):
    n = rhs.shape[-1]
    for s in range(0, n, half):
        e = min(s + half, n)
        nc.tensor.matmul(out[:, s:e], lhsT, rhs[:, s:e], start=start, stop=stop)


def _build_nc():
    nc = bacc.Bacc(None, debug=False)

    x_d = nc.declare_dram_parameter("xc", [CIN, B_C * NPIX], F32, isOutput=False)
    wq_d = nc.declare_dram_parameter("wqT", [CIN, CO], F32, isOutput=False)
    wk_d = nc.declare_dram_parameter("wkT", [CIN, CO], F32, isOutput=False)
    wv_d = nc.declare_dram_parameter("wvT", [CIN, CO], F32, isOutput=False)
    relcol_d = nc.declare_dram_parameter("relcol", [128, KK], F32, isOutput=False)
    sel_d = nc.declare_dram_parameter("sel128", [128, 128], F32, isOutput=False)
    out_d = nc.declare_dram_parameter("out", [128, NQ], BF16, isOutput=True)

    with tile.TileContext(nc) as tc:
        with (
            tc.tile_pool(name="const", bufs=1) as constp,
            tc.tile_pool(name="proj", bufs=1) as projp,
            tc.tile_pool(name="psA", bufs=2, space="PSUM") as psA,
            tc.tile_pool(name="ptile", bufs=3) as ptile,
            tc.tile_pool(name="atile", bufs=3) as atile,
            tc.tile_pool(name="otile", bufs=1) as otile,
        ):
            sel_sb = constp.tile([128, 128], F32, tag="sel")
            nc.gpsimd.dma_start(out=sel_sb[:], in_=sel_d[:])
            relcol_sb = constp.tile([128, KK], F32, tag="rel")
            nc.gpsimd.dma_start(out=relcol_sb[:], in_=relcol_d[:])

            x_sb = projp.tile([CIN, B_C * NPIX], F32, tag="x")
            nc.sync.dma_start(out=x_sb[:], in_=x_d[:])
            wq_sb = constp.tile([CIN, CO], F32, tag="w1")
            wk_sb = constp.tile([CIN, CO], F32, tag="w2")
            wv_sb = constp.tile([CIN, CO], F32, tag="w3")
            nc.gpsimd.dma_start(out=wq_sb[:], in_=wq_d[:])
            nc.gpsimd.dma_start(out=wk_sb[:], in_=wk_d[:])
            nc.gpsimd.dma_start(out=wv_sb[:], in_=wv_d[:])

            q_sb = projp.tile([128, NQ], F32, tag="q")
            k_sb = projp.tile([128, 2, HP, WP], F32, tag="k")
            v_sb = projp.tile([128, 2, HP, WP], F32, tag="v")
            nc.vector.memset(k_sb[:], 0.0)
            nc.vector.memset(v_sb[:], 0.0)

            for (wt, dst, padded) in ((wq_sb, q_sb, False), (wk_sb, k_sb, True),
                                      (wv_sb, v_sb, True)):
                for e in range(2):
                    for ch in range(2):
                        pp = psA.tile([128, NPIX], F32, tag="sps")
                        img = 2 * e + ch
                        _mm(nc, pp[64 * e:64 * e + 64, :], wt[:],
                            x_sb[:, NPIX * img: NPIX * img + NPIX],
                            start=True, stop=True)
                        src = pp[64 * e:64 * e + 64].rearrange(
                            "p (h w) -> p h w", h=H)
                        if padded:
                            dv = dst[64 * e:64 * e + 64, ch,
                                     PAD:PAD + H, PAD:PAD + W]
                        else:
                            dv = dst[64 * e:64 * e + 64,
                                     NPIX * ch:NPIX * ch + NPIX].rearrange(
                                "p (h w) -> p h w", h=H)
                        nc.scalar.activation(
                            out=dv, in_=src,
                            func=mybir.ActivationFunctionType.Copy)

            o_sb = otile.tile([128, NQ], F32, tag="o")
            d_sb = otile.tile([128, NQ], F32, tag="d")
            dm_sb = otile.tile([128, NQ], F32, tag="dm")
            for t in range(KK):
                dh, dw = t // K, t % K
                a_t = atile.tile([128, NQ], F32, tag="at")
                p2_t = ptile.tile([128, NQ], F32, tag="p2t")
                for qc in range(2):
                    kview = k_sb[:, qc, dh:dh + H, dw:dw + W]
                    vview = v_sb[:, qc, dh:dh + H, dw:dw + W]
                    q_chunk = q_sb[:, NPIX * qc:NPIX * qc + NPIX].rearrange(
                        "p (h w) -> p h w", h=H)
                    p_t = ptile.tile([128, H, W], F32, tag="pt")
                    nc.vector.scalar_tensor_tensor(
                        out=p_t[:], in0=kview, scalar=relcol_sb[:, t:t + 1],
                        in1=q_chunk,
                        op0=mybir.AluOpType.add, op1=mybir.AluOpType.mult)
                    s_ps = psA.tile([128, NPIX], F32, tag="sps")
                    _mm(nc, s_ps[:], sel_sb[:],
                        p_t.rearrange("p h w -> p (h w)"),
                        start=True, stop=True)
                    a_c = a_t[:, NPIX * qc:NPIX * qc + NPIX]
                    nc.scalar.activation(out=a_c, in_=s_ps[:],
                                         func=mybir.ActivationFunctionType.Exp)
                    nc.vector.tensor_tensor(
                        out=p2_t[:, NPIX * qc:NPIX * qc + NPIX].rearrange(
                            "p (h w) -> p h w", h=H),
                        in0=a_c.rearrange("p (h w) -> p h w", h=H),
                        in1=vview, op=mybir.AluOpType.mult)
                if t == 0:
                    nc.gpsimd.tensor_copy(out=d_sb[:], in_=a_t[:])
                    nc.vector.tensor_copy(out=o_sb[:], in_=p2_t[:])
                else:
                    nc.gpsimd.tensor_tensor(out=d_sb[:], in0=d_sb[:],
                                            in1=a_t[:],
                                            op=mybir.AluOpType.add)
                    nc.vector.tensor_tensor(out=o_sb[:], in0=o_sb[:],
                                            in1=p2_t[:],
                                            op=mybir.AluOpType.add)
            with nc.allow_low_precision(reason="softmax denom recip, tol 2e-2"):
                nc.vector.reciprocal(out=dm_sb[:], in_=d_sb[:])

            of = otile.tile([128, NQ], BF16, tag="ofin")
            nc.vector.tensor_tensor(out=of[:], in0=o_sb[:], in1=dm_sb[:],
                                    op=mybir.AluOpType.mult)
            nc.sync.dma_start(out=out_d[:], in_=of[:])

    nc.finalize()
    return nc


def _make_host_consts(w_q, w_k, w_v, rel_h, rel_w, current_val):
    wqT = np.ascontiguousarray(np.asarray(w_q, np.float32).T).astype(np.float32)
    wkT = np.ascontiguousarray(np.asarray(w_k, np.float32).T).astype(np.float32)
    wvT = np.ascontiguousarray(np.asarray(w_v, np.float32).T).astype(np.float32)
    rh = np.asarray(rel_h, np.float32).reshape(32, K)   # [c<32, dh]
    rw = np.asarray(rel_w, np.float32).reshape(32, K)   # [c>=32, dw]
    relcol = np.zeros((128, KK), np.float32)
    for t in range(KK):
        dh, dw = t // K, t % K
        col = np.concatenate([rh[:, dh], rw[:, dw]])
        relcol[:, t] = np.tile(col, 2)
    ee = np.arange(128) // 64
    cc = np.arange(128) % 64
    gg = cc // (CO // G)
    sel128 = ((ee[:, None] == ee[None, :]) &
              (gg[:, None] == gg[None, :])).astype(np.float32)

    MAXSZ = W // 2
    template = np.linspace(1.0 - MAXSZ, 0.0, MAXSZ).astype(np.float32)
    om = (template[None, :]
          + np.asarray(current_val, np.float32) * MAXSZ) / 3.0 + 1.0
    om = np.clip(om, 0.0, 1.0)
    i = np.arange(W)
    r = np.minimum(i, W - 1 - i)
    top = i <= (W - 1 - i)
    lo = np.where(top, r, r + 1)
    hi = W - 1 - r
    c = np.arange(W)
    in_ring = (c[None, :] >= lo[:, None]) & (c[None, :] <= hi[:, None])
    vals = om[:, r]
    mask = np.where(in_ring[None], vals[:, :, None], 1.0).astype(np.float32)
    return dict(wqT=wqT, wkT=wkT, wvT=wvT, relcol=relcol,
                sel128=sel128), mask


_NC = None


def _get_nc():
    global _NC
    if _NC is None:
        _NC = _build_nc()
    return _NC


_SHARDED = None


def _get_sharded():
    """Build the shard_map executable ONCE (mirrors the multi-core branch of
    bass2jax.run_bass_via_pjrt, including the partition_id operand the NEFF
    requires) so repeat kernel() calls skip the per-call jit rebuild."""
    global _SHARDED
    if _SHARDED is not None:
        return _SHARDED
    import jax
    from jax.sharding import Mesh, PartitionSpec
    from jax.experimental.shard_map import shard_map
    from concourse import bass2jax
    import concourse.mybir as mybir

    nc = _get_nc()
    bass2jax.install_neuronx_cc_hook()
    partition_name = (nc.partition_id_tensor.name
                      if nc.partition_id_tensor else None)
    in_names, out_names, out_avals, zero_outs = [], [], [], []
    for alloc in nc.m.functions[0].allocations:
        if not isinstance(alloc, mybir.MemoryLocationSet):
            continue
        name = alloc.memorylocations[0].name
        if alloc.kind == "ExternalInput":
            if name != partition_name:
                in_names.append(name)
        elif alloc.kind == "ExternalOutput":
            shape = tuple(alloc.tensor_shape)
            dtype = mybir.dt.np(alloc.dtype)
            out_names.append(name)
            out_avals.append(jax.core.ShapedArray(shape, dtype))
            zero_outs.append(np.zeros(shape, dtype))
    n_params = len(in_names)
    n_outs = len(out_avals)
    all_names = list(in_names) + list(out_names)
    if partition_name is not None:
        all_names.append(partition_name)
    donate = tuple(range(n_params, n_params + n_outs))

    def _body(*args):
        operands = list(args)
        if partition_name is not None:
            operands.append(bass2jax.partition_id_tensor())
        outs = bass2jax._bass_exec_p.bind(
            *operands,
            out_avals=tuple(out_avals),
            in_names=tuple(all_names),
            out_names=tuple(out_names),
            lowering_input_output_aliases=(),
            sim_require_finite=True,
            sim_require_nnan=True,
            nc=nc,
        )
        return tuple(outs)

    devices = jax.devices()[:N_CORES]
    mesh = Mesh(np.asarray(devices), ("core",))
    sharded = jax.jit(
        shard_map(_body, mesh=mesh,
                  in_specs=(PartitionSpec("core"),) * (n_params + n_outs),
                  out_specs=(PartitionSpec("core"),) * n_outs,
                  check_rep=False),
        donate_argnums=donate, keep_unused=True)
    _SHARDED = (sharded, in_names, zero_outs)
    return _SHARDED


def kernel(x, w_q, w_k, w_v, rel_h, rel_w, current_val):
    x = np.asarray(x, np.float32)
    consts, mask = _make_host_consts(w_q, w_k, w_v, rel_h, rel_w, current_val)
    xcs = []
    for i in range(N_CORES):
        x4 = x[B_C * i:B_C * i + B_C]
        xcs.append(np.ascontiguousarray(
            x4.transpose(1, 0, 2, 3).reshape(CIN, -1)).astype(np.float32))

    try:
        sharded, in_names, zero_outs = _get_sharded()
        per_core = [dict(consts, xc=xcs[i]) for i in range(N_CORES)]
        concat_in = [np.concatenate([per_core[c][n] for c in range(N_CORES)],
                                    axis=0) for n in in_names]
        concat_zeros = [np.zeros((N_CORES * z.shape[0], *z.shape[1:]), z.dtype)
                        for z in zero_outs]
        out_arrs = sharded(*concat_in, *concat_zeros)
        out_all = np.asarray(out_arrs[0]).reshape(N_CORES, 128, NQ)
    except Exception:
        # fallback: library per-call path
        from concourse.bass_utils import run_bass_kernel_spmd
        nc = _get_nc()
        in_maps = [dict(consts, xc=xcs[i]) for i in range(N_CORES)]
        res = run_bass_kernel_spmd(nc, in_maps, core_ids=list(range(N_CORES)))
        out_all = np.stack([np.asarray(res.results[i]["out"])
                            for i in range(N_CORES)])

    outs = []
    for i in range(N_CORES):
        o = np.asarray(out_all[i], np.float32).reshape(2, 64, 2, H, W)
        outs.append(o.transpose(0, 2, 1, 3, 4).reshape(B_C, CO, H, W))
    full = np.ascontiguousarray(np.concatenate(outs, axis=0)).astype(np.float32)
    # reference returns grouped shape [B, G, CPG, H, W]; adaptive mask on host
    out5 = full.reshape(B, G, CO // G, H, W)
    return (out5 * mask[None, :, None, :, :]).astype(np.float32)


# Build (and, via the persistent compile cache, warm) at import so that the
# first timed kernel() call does not pay IR construction.
_get_nc()
